# revision 5
# baseline (speedup 1.0000x reference)
"""MDCA loss kernel v2 for Trainium2, 8 NeuronCores, data-parallel over batch.

reference:
    counts[c]   = histogram(target) ; avg_count = counts/B
    avg_conf    = mean(logits, axis=1)            # [E, C]
    loss[e]     = mean_c |avg_conf[e,c] - avg_count[c]|

v2 changes vs baseline (66.4us):
  - logits ride ONLY the two HWDGE rings (sync=qSP, scalar=qAct), 8.0 MB
    each, interleaved triggers; SWDGE/gpsimd only carries the 4KB target
    (baseline's gpsimd queue started 5us late and dragged the window)
  - tapered chunks: 2MB (4 rows) for e0-e2, 1MB (2 rows) + 2x0.5MB
    (1 row) for e3 -> the drain tail does a 0-or-1.1us fold instead of 2.2
  - logits dram tensor is float32r, so the two final 0.5MB chunks matmul
    straight out of the DMA tile in f32r fast mode (no DVE fold at all)
  - histogram matmuls (-1 weights, bf16 one-hots) accumulate into the SAME
    PSUM banks as the conf matmuls -> psum = sum_conf - count directly; the
    baseline's 2 PSUM copies + 2 adds in the tail disappear
  - final PSUM->SBUF copies run on vector (h0) and gpsimd (h1) in parallel,
    each half DMA'd out on its own HWDGE ring as soon as it's ready
  - host sums the 8 per-core partials and takes |.|/(B*C) -> loss[4]
"""

import os
import sys

for _p in ("/opt/trn_rl_repo", "/root/.axon_site/_ro/trn_rl_repo"):
    if os.path.isdir(_p) and _p not in sys.path:
        sys.path.insert(0, _p)

import numpy as np

import concourse.bass as bass
import concourse.bacc as bacc
import concourse.tile as tile
import concourse.mybir as mybir
from concourse.bass_utils import run_bass_kernel_spmd

E, B, C = 4, 8192, 1000
N_CORES = 8
BS = B // N_CORES          # 1024 batch rows per core
GP = 8                     # rows folded per partition (BS = 128 * GP)
CH = C // 2                # 500, C half per PSUM bank
F32 = mybir.dt.float32
F32R = mybir.dt.float32r
BF16 = mybir.dt.bfloat16

# (exit, row_start, row_end, queue, col_half) — queue 0=sync ring,
# 1=scalar ring; col_half -1 = both halves, 0/1 = that C-half only.
# 8.0 MB per ring; chunks taper to 0.5MB at the end (single-row chunks
# matmul straight from the DMA tile, no fold) so the drain is short.
# Taper below 0.5MB measured WORSE: sub-0.5MB transfers are descriptor-
# dominated and extend the DMA window more than they shorten the drain.
CHUNKS = [
    (0, 0, 4, 0, -1), (0, 4, 8, 1, -1),
    (1, 0, 4, 0, -1), (1, 4, 8, 1, -1),
    (2, 0, 4, 0, -1), (2, 4, 8, 1, -1),
    (3, 0, 2, 0, -1), (3, 2, 4, 1, -1),
    (3, 4, 6, 1, -1), (3, 6, 7, 0, -1), (3, 7, 8, 1, -1),
]


def build_nc():
    nc = bacc.Bacc(
        "TRN2",
        target_bir_lowering=False,
        debug=False,
        enable_asserts=False,
        num_devices=N_CORES,
    )

    # float32r = same bits as f32; numpy binding maps it to np.float32, and
    # it lets the PE run 1 cyc/row on tiles loaded straight from DRAM.
    logits = nc.dram_tensor("logits", [E, BS, C], F32R, kind="ExternalInput")
    # host pre-arranges the target shard as [128, GP] float32 (exact ints)
    target = nc.dram_tensor("target_f", [128, GP], F32, kind="ExternalInput")
    part_out = nc.dram_tensor("part", [E, C], F32, kind="ExternalOutput")

    with tile.TileContext(nc) as tc:
        with (
            tc.tile_pool(name="const", bufs=1) as const,
            tc.tile_pool(name="ld", bufs=1) as ld,
            tc.tile_pool(name="work", bufs=3) as work,
            tc.tile_pool(name="oh", bufs=2) as oh,
            tc.tile_pool(name="out", bufs=1) as outp,
            tc.tile_pool(name="psum", bufs=1, space=bass.MemorySpace.PSUM) as psum,
        ):
            dma_q = [nc.sync, nc.scalar]

            # 0) 4KB target first on the sync ring (HWDGE, FIFO ahead of the
            # logits chunks — lands ~1us in, unblocking the one-hot chain
            # early, and keeps SWDGE fully idle for the whole kernel)
            tgt_sb = const.tile([128, GP], F32, tag="tgt")
            nc.sync.dma_start(out=tgt_sb[:], in_=target[:])

            # 1) logits chunk DMAs first — HWDGE triggers cost ~0.7us each on
            # the issuing engine, so getting them queued is the whole game.
            ld_tiles = []
            for idx, (e, r0, r1, q, hq) in enumerate(CHUNKS):
                nr = r1 - r0
                src = logits[e].rearrange("(p i) c -> p i c", i=GP)
                if hq < 0:
                    t = ld.tile(
                        [128, nr * C], F32R, tag=f"ld{idx}", name=f"ld{idx}"
                    )
                    dma_q[q].dma_start(
                        out=t.rearrange("p (i c) -> p i c", i=nr),
                        in_=src[:, r0:r1, :],
                    )
                else:
                    t = ld.tile([128, CH], F32R, tag=f"ld{idx}", name=f"ld{idx}")
                    dma_q[q].dma_start(
                        out=t[:], in_=src[:, r0, hq * CH : (hq + 1) * CH]
                    )
                ld_tiles.append(t)

            # 2) iota on gpsimd (target DMA already issued above)
            iota_f = const.tile([128, C], F32, tag="iota")
            nc.gpsimd.iota(
                iota_f[:],
                pattern=[[1, C]],
                base=0,
                channel_multiplier=0,
                allow_small_or_imprecise_dtypes=True,
            )

            # 3) constants: selector weights (ones in column e of block e) in
            # f32r, -1 weights for the histogram rows
            sels_f = const.tile([128, 4 * E], F32, tag="sels_f")
            nc.vector.memset(sels_f[:], 0.0)
            for e in range(E):
                nc.vector.memset(sels_f[:, 4 * e + e : 4 * e + e + 1], 1.0)
            sels = const.tile([128, 4 * E], F32R, tag="sels")
            nc.vector.tensor_copy(sels[:], sels_f[:])
            neg1 = const.tile([128, E], BF16, tag="neg1")
            nc.vector.memset(neg1[:], -1.0)

            psum_acc = [
                psum.tile([E, CH], F32, tag=f"pacc{h}", name=f"pacc{h}")
                for h in range(2)
            ]

            # 4) histogram one-hots + matmuls: first writers of the PSUM
            # banks (start=True), so they must be scheduled before any conf
            # matmul — they only need iota+target, both done early.
            for i in range(GP):
                onehot = oh.tile([128, C], BF16, tag="onehot")
                nc.vector.tensor_scalar(
                    onehot[:],
                    iota_f[:],
                    tgt_sb[:, i : i + 1],
                    None,
                    mybir.AluOpType.is_equal,
                )
                for h in range(2):
                    nc.tensor.matmul(
                        psum_acc[h][:],
                        neg1[:],
                        onehot[:, h * CH : (h + 1) * CH],
                        start=(i == 0),
                        stop=False,
                    )

            # 5) per-chunk: fold row-pairs on DVE (multi-row chunks), then
            # f32r matmuls with the selector fold the partitions into
            # PSUM[4, CH], accumulating on top of -count. Each bank's stop
            # lands on its final chunk (h1 one chunk before h0).
            last_of_bank = {0: None, 1: None}
            for idx, (e, r0, r1, q, hq) in enumerate(CHUNKS):
                nr = r1 - r0
                halves = [hq] if hq >= 0 else [0, 1]
                for h in halves:
                    last_of_bank[h] = (idx, (nr // 2 or 1) - 1)
            for idx, (e, r0, r1, q, hq) in enumerate(CHUNKS):
                nr = r1 - r0
                t = ld_tiles[idx]
                if nr > 1:
                    hw = (nr // 2) * C
                    f = work.tile([128, hw], F32R, tag="fold", name=f"f{idx}")
                    nc.vector.tensor_add(f[:], t[:, :hw], t[:, hw:])
                    ngroups = nr // 2
                else:
                    f = t
                    ngroups = 1
                for g in range(ngroups):
                    if hq >= 0:
                        nc.tensor.matmul(
                            psum_acc[hq][:],
                            sels[:, 4 * e : 4 * e + 4],
                            f[:, 0:CH],
                            start=False,
                            stop=(last_of_bank[hq] == (idx, g)),
                        )
                        continue
                    # h=1 first on the final chunk so bank1 closes early and
                    # its ACT-copy/out-DMA chain overlaps bank0's last matmul
                    horder = (1, 0) if idx == len(CHUNKS) - 1 else (0, 1)
                    for h in horder:
                        nc.tensor.matmul(
                            psum_acc[h][:],
                            sels[:, 4 * e : 4 * e + 4],
                            f[:, g * C + h * CH : g * C + (h + 1) * CH],
                            start=False,
                            stop=(last_of_bank[h] == (idx, g)),
                        )

            # 6) two independent drain chains: bank1 closes first ->
            # ACT identity-copy -> out DMA on the scalar ring, while bank0's
            # last chunk is still landing; bank0 -> DVE copy -> sync ring.
            # (gpsimd cannot read PSUM, hence DVE + ACT.)
            part_sb = outp.tile([E, C], F32, tag="part")
            nc.scalar.activation(
                part_sb[:, CH:C],
                psum_acc[1][:],
                mybir.ActivationFunctionType.Copy,
            )
            nc.scalar.dma_start(out=part_out[:, CH:C], in_=part_sb[:, CH:C])
            nc.vector.tensor_copy(part_sb[:, 0:CH], psum_acc[0][:])
            nc.sync.dma_start(out=part_out[:, 0:CH], in_=part_sb[:, 0:CH])

    nc.compile()
    return nc


_NC_CACHE = {}

# kept for test.py compatibility (host finish; no device collective)
DEVICE_FINISH = False


def _get_nc(device_finish: bool = False):
    if "nc" not in _NC_CACHE:
        _NC_CACHE["nc"] = build_nc()
    return _NC_CACHE["nc"]


def make_in_maps(logits: np.ndarray, target: np.ndarray):
    logits = np.ascontiguousarray(logits, dtype=np.float32)
    target = np.asarray(target)
    in_maps = []
    for c in range(N_CORES):
        lg = logits[:, c * BS : (c + 1) * BS, :]
        tg = target[c * BS : (c + 1) * BS].astype(np.float32).reshape(128, GP)
        in_maps.append({"logits": np.ascontiguousarray(lg), "target_f": tg})
    return in_maps


def kernel(logits: np.ndarray, target: np.ndarray) -> np.ndarray:
    nc = _get_nc()
    in_maps = make_in_maps(logits, target)
    res = run_bass_kernel_spmd(nc, in_maps, core_ids=list(range(N_CORES)))
    parts = sum(np.asarray(r["part"], dtype=np.float64) for r in res.results)
    return (np.abs(parts).sum(axis=1) / (B * C)).astype(np.float32)
